# revision 28
# baseline (speedup 1.0000x reference)
"""Trainium2 Bass kernel for nn_DeepseekCompressor (scatter_memory).

Computation: kv_score = x @ W.T; score half += ape[positions % 128];
rows scattered into a paged state cache at slot_mapping.

Sharding (8 NeuronCores, data-parallel over tokens):
  - x, positions, slot_mapping sharded by token (2048 tokens/core).
  - W, ape replicated (host pre-transposes W and casts to bf16; ape rows
    pre-gathered per token in f32 on host).
  - state_cache sharded by block: with the contiguous slot_mapping each
    core's tokens land in its own contiguous range of cache rows, and the
    untouched half of the cache is pass-through-copied, one slice per core.

Device kernel per core: bf16 GEMM [2048,7168]@[7168,1024] -> f32 PSUM,
ape added at PSUM eviction (free: replaces the copy), contiguous DMA
scatter into the cache rows, DRAM->DRAM pass-through of untouched blocks.
"""

import os
import sys
import types
from contextlib import ExitStack

if "/opt/trn_rl_repo" not in sys.path:
    sys.path.insert(0, "/opt/trn_rl_repo")

import numpy as np
import ml_dtypes

import concourse.bass as bass
import concourse.tile as tile
from concourse import bacc, mybir
from concourse.bass_utils import run_bass_kernel_spmd

NCORES = 8
T = 16384          # tokens
H = 7168           # hidden
D2 = 1024          # 2 * state_width
D = 512            # state_width
CR = 128           # compress ratio (ape rows)
TC = T // NCORES   # tokens per core
P = 128
NK = H // P        # k-chunks of 128
MT = TC // P       # m-tiles per core (16)
GM = 4             # m-tiles per PSUM group
NG = MT // GM      # groups (4)
NB = 4096 * 8      # flat cache rows

BF16 = ml_dtypes.bfloat16

LAST_RESULTS = None
_PROGRAM = None


def _install_ntff_hook():
    """Make trace=True work under axon: register the NTFF profile hook that
    the image's antenv is missing, and stub the (egress-only) artifact
    upload. No-ops if anything is unavailable."""
    try:
        import antenv
        if "antenv.axon_hooks" not in sys.modules:
            mod = types.ModuleType("antenv.axon_hooks")
            _state = {"hook": None}
            mod.set_axon_ntff_profile_hook = lambda h: _state.__setitem__("hook", h)
            mod.get_axon_ntff_profile_hook = lambda: _state["hook"]
            sys.modules["antenv.axon_hooks"] = mod
            antenv.axon_hooks = mod
            from trn_agent_boot.trn_boot import _ntff_profile_via_ctypes
            mod.set_axon_ntff_profile_hook(
                _ntff_profile_via_ctypes("/opt/axon/libaxon_pjrt.so")
            )
        import concourse.bass_utils as _bu
        _bu.upload_artifacts = lambda tmpdir: tmpdir
    except Exception:
        pass


def _build_program():
    nc = bacc.Bacc(None, target_bir_lowering=False)
    # x pre-tiled on host: [group, k-quad, 128 k, 4 chunks, 512 tokens]; each
    # (group, k-quad) tile is a contiguous 512KB block with 4KB per-partition
    # descriptors, so x DMAs run at HBM line rate
    xT = nc.declare_dram_parameter(
        "xT", [NG, NK // 4, P, 4, GM * P], mybir.dt.bfloat16, isOutput=False
    )
    # W pre-tiled on host: [k-quad, 128 k, 4 chunks * 1024], contiguous 1MB
    # tiles with 8KB per-partition descriptors
    wT = nc.declare_dram_parameter(
        "wT", [NK // 4, P, 4 * D2], mybir.dt.bfloat16, isOutput=False
    )
    ape_rows = nc.declare_dram_parameter("ape_rows", [TC, D], mybir.dt.float32, isOutput=False)
    cache_in = nc.declare_dram_parameter("cache_in", [TC, D2], mybir.dt.float32, isOutput=False)
    out_new = nc.declare_dram_parameter(
        "out_new", [TC, D2], mybir.dt.float32, isOutput=True
    )
    out_pass = nc.declare_dram_parameter(
        "out_pass", [TC, D2], mybir.dt.float32, isOutput=True
    )

    with tile.TileContext(nc) as tc, ExitStack() as ctx:
        wpool = ctx.enter_context(tc.tile_pool(name="w", bufs=NK // 4))
        xpool = ctx.enter_context(tc.tile_pool(name="x", bufs=8))
        opool = ctx.enter_context(tc.tile_pool(name="o", bufs=3))
        apool = ctx.enter_context(tc.tile_pool(name="ape", bufs=2 * GM))
        ppool = ctx.enter_context(tc.tile_pool(name="ps", bufs=8, space="PSUM"))

        # W resident in SBUF: 14 tiles of [128, 4096] bf16 (4 k-chunks each),
        # each a contiguous 1MB DMA. W and x quads are striped across both
        # HWDGE rings in consumption order (each ring carries ~half of W plus
        # ~half of x, ~111GB/s demand each) so early delivery keeps pace with
        # the PE. The first W quad is split per-chunk so matmul 0 starts fast.
        wt = [
            wpool.tile([P, 4 * D2], mybir.dt.bfloat16, tag="w", name=f"w{j}")
            for j in range(NK // 4)
        ]
        for c in range(4):
            nc.sync.dma_start(
                wt[0][:, c * D2:(c + 1) * D2], wT[0, :, c * D2:(c + 1) * D2]
            )

        def w_load(j):
            eng = nc.sync if j % 2 == 0 else nc.scalar
            eng.dma_start(wt[j][:], wT[j])

        # pass-through tile ranges per group (emitted after groups 0..2)
        pass_splits = [(0, 6), (6, 11), (11, 16)]

        # scratch operand for PE warmup matmuls (zeroed: uninitialized SBUF
        # reads fault the exec unit)
        warm_sb = opool.tile([P, 64], mybir.dt.bfloat16, tag="warm", name="warm_sb")
        nc.vector.memset(warm_sb[:], 0.0)

        for g in range(NG):
            psums = [
                ppool.tile([P, D], mybir.dt.float32, tag="acc", name=f"acc{g}_{i}")
                for i in range(GM * 2)
            ]
            if g == 0:
                # Keep the PE busy while the first W/x DMAs are in flight:
                # HAM un-throttles after ~3.4us of sustained activity, so the
                # first real matmuls run at 2.4GHz instead of 1.2GHz. These
                # write psum bank 0, which the first start=True matmul resets.
                for i in range(100):
                    nc.tensor.matmul(
                        psums[0][0:64, 0:64], warm_sb[:, 0:64], warm_sb[:, 0:64],
                        start=True, stop=True,
                    )
            for A in range(NK // 4):
                # x quad-chunk [128 k, 4, 512 tokens]; rings alternate per
                # quad; very first quad split per-chunk for first-MM latency
                xt = xpool.tile([P, 4, GM * P], mybir.dt.bfloat16, tag="x")
                x_eng = nc.scalar if A % 2 == 0 else nc.sync
                if g == 0 and A == 0:
                    for c in range(4):
                        nc.scalar.dma_start(xt[:, c, :], xT[g, A, :, c, :])
                else:
                    x_eng.dma_start(xt[:], xT[g, A])
                if g == 0 and A + 1 < NK // 4:
                    # next W quad, emitted in consumption order on its ring
                    w_load(A + 1)
                for c in range(4):
                    a = 4 * A + c
                    for mi in range(GM):
                        lhsT = xt[:, c, mi * P:(mi + 1) * P]
                        nc.tensor.matmul(
                            psums[2 * mi][:], lhsT, wt[A][:, c * D2:c * D2 + D],
                            start=(a == 0), stop=(a == NK - 1),
                        )
                        nc.tensor.matmul(
                            psums[2 * mi + 1][:], lhsT, wt[A][:, c * D2 + D:(c + 1) * D2],
                            start=(a == 0), stop=(a == NK - 1),
                        )

            # ape rows arrive on the (otherwise idle) SWDGE ring well before
            # eviction; emitted late so Q0 is silent during the startup window
            apes = []
            for mi in range(GM):
                m = g * GM + mi
                at = apool.tile([P, D], mybir.dt.float32, tag="ape", name=f"ape{g}_{mi}")
                nc.gpsimd.dma_start(at[:], ape_rows[m * P:(m + 1) * P, :])
                apes.append(at)

            for mi in range(GM):
                m = g * GM + mi
                ot = opool.tile([P, D2], mybir.dt.float32, tag="o", name=f"ot{g}_{mi}")
                nc.vector.tensor_copy(ot[:, 0:D], psums[2 * mi][:])
                nc.vector.tensor_add(ot[:, D:D2], psums[2 * mi + 1][:], apes[mi][:])
                # sync ring is idle once W is resident; stores complete fast
                nc.sync.dma_start(out_new[m * P:(m + 1) * P, :], ot[:])

            # pass-through of untouched cache blocks, bounced through the
            # eviction tile pool: the slot dependency chains each piece
            # behind this group's stores, keeping it out of the startup
            # window without fake timing hints
            if g < 3:
                for i in range(*pass_splits[g]):
                    bt = opool.tile([P, D2], mybir.dt.float32, tag="o",
                                    name=f"pt{g}_{i}")
                    nc.gpsimd.dma_start(bt[:], cache_in[i * P:(i + 1) * P, :])
                    nc.gpsimd.dma_start(out_pass[i * P:(i + 1) * P, :], bt[:])

    nc.compile()
    return nc


def _get_program():
    global _PROGRAM
    if _PROGRAM is None:
        _install_ntff_hook()
        _PROGRAM = _build_program()
    return _PROGRAM


def kernel(x, W, ape, state_cache, positions, slot_mapping, block_size=8):
    global LAST_RESULTS
    x = np.asarray(x)
    W = np.asarray(W)
    ape = np.asarray(ape)
    state_cache = np.asarray(state_cache)
    positions = np.asarray(positions)
    slot_mapping = np.asarray(slot_mapping)

    assert x.shape == (T, H) and W.shape == (D2, H) and ape.shape == (CR, D)
    assert state_cache.shape == (4096, 8, D2)

    # host-side input prep (layout/sharding glue)
    # W^T repacked to [14, 128, 4096]: tile j partition p holds rows
    # (4j+c)*128+p of W^T for c=0..3
    wTb = np.ascontiguousarray(
        W.astype(BF16).T.reshape(NK // 4, 4, P, D2).transpose(0, 2, 1, 3)
        .reshape(NK // 4, P, 4 * D2)
    )
    xb = x.astype(BF16)                                     # [T, H] bf16
    pos_mod = (positions.astype(np.int64) % CR).astype(np.int64)
    ape_rows_full = np.ascontiguousarray(ape[pos_mod])      # [T, D] f32
    cache_flat = state_cache.reshape(NB, D2)

    fast = (
        slot_mapping.shape == (T,)
        and np.array_equal(slot_mapping, np.arange(T, dtype=slot_mapping.dtype))
    )

    zeros_cache = None if fast else np.zeros((TC, D2), np.float32)
    in_maps = []
    for c in range(NCORES):
        t0, t1 = c * TC, (c + 1) * TC
        in_maps.append({
            # [NG, 14, 128, 4, 512]: per-(group, k-quad) contiguous tiles
            "xT": np.ascontiguousarray(
                xb[t0:t1].reshape(NG, GM * P, NK // 4, 4, P)
                .transpose(0, 2, 4, 3, 1)
            ),
            "wT": wTb,
            "ape_rows": ape_rows_full[t0:t1],
            "cache_in": (
                np.ascontiguousarray(cache_flat[T + t0:T + t1]).astype(
                    np.float32, copy=False
                )
                if fast else zeros_cache
            ),
        })

    nc = _get_program()
    trace = os.environ.get("KERNEL_TRACE", "0") == "1"
    res = run_bass_kernel_spmd(nc, in_maps, list(range(NCORES)), trace=trace)
    LAST_RESULTS = res

    out_flat = np.empty((NB, D2), np.float32)
    if fast:
        for c in range(NCORES):
            t0, t1 = c * TC, (c + 1) * TC
            out_flat[t0:t1] = np.asarray(res.results[c]["out_new"])
            out_flat[T + t0:T + t1] = np.asarray(res.results[c]["out_pass"])
    else:
        # general slot_mapping: device computes new_vals; host scatters
        out_flat[:] = cache_flat
        new_vals = np.concatenate(
            [np.asarray(res.results[c]["out_new"]) for c in range(NCORES)], axis=0
        )
        ok = (slot_mapping >= 0) & (slot_mapping < NB)
        out_flat[slot_mapping[ok]] = new_vals[ok]
    return out_flat.reshape(4096, 8, D2)


# revision 29
# speedup vs baseline: 1.0003x; 1.0003x over previous
"""Trainium2 Bass kernel for nn_DeepseekCompressor (scatter_memory).

Computation: kv_score = x @ W.T; score half += ape[positions % 128];
rows scattered into a paged state cache at slot_mapping.

Sharding (8 NeuronCores, data-parallel over tokens):
  - x, positions, slot_mapping sharded by token (2048 tokens/core).
  - W, ape replicated (host pre-transposes W and casts to bf16; ape rows
    pre-gathered per token in f32 on host).
  - state_cache sharded by block: with the contiguous slot_mapping each
    core's tokens land in its own contiguous range of cache rows, and the
    untouched half of the cache is pass-through-copied, one slice per core.

Device kernel per core: bf16 GEMM [2048,7168]@[7168,1024] -> f32 PSUM,
ape added at PSUM eviction (free: replaces the copy), contiguous DMA
scatter into the cache rows, DRAM->DRAM pass-through of untouched blocks.
"""

import os
import sys
import types
from contextlib import ExitStack

if "/opt/trn_rl_repo" not in sys.path:
    sys.path.insert(0, "/opt/trn_rl_repo")

import numpy as np
import ml_dtypes

import concourse.bass as bass
import concourse.tile as tile
from concourse import bacc, mybir
from concourse.bass_utils import run_bass_kernel_spmd

NCORES = 8
T = 16384          # tokens
H = 7168           # hidden
D2 = 1024          # 2 * state_width
D = 512            # state_width
CR = 128           # compress ratio (ape rows)
TC = T // NCORES   # tokens per core
P = 128
NK = H // P        # k-chunks of 128
MT = TC // P       # m-tiles per core (16)
GM = 4             # m-tiles per PSUM group
NG = MT // GM      # groups (4)
NB = 4096 * 8      # flat cache rows

BF16 = ml_dtypes.bfloat16

LAST_RESULTS = None
_PROGRAM = None


def _install_ntff_hook():
    """Make trace=True work under axon: register the NTFF profile hook that
    the image's antenv is missing, and stub the (egress-only) artifact
    upload. No-ops if anything is unavailable."""
    try:
        import antenv
        if "antenv.axon_hooks" not in sys.modules:
            mod = types.ModuleType("antenv.axon_hooks")
            _state = {"hook": None}
            mod.set_axon_ntff_profile_hook = lambda h: _state.__setitem__("hook", h)
            mod.get_axon_ntff_profile_hook = lambda: _state["hook"]
            sys.modules["antenv.axon_hooks"] = mod
            antenv.axon_hooks = mod
            from trn_agent_boot.trn_boot import _ntff_profile_via_ctypes
            mod.set_axon_ntff_profile_hook(
                _ntff_profile_via_ctypes("/opt/axon/libaxon_pjrt.so")
            )
        import concourse.bass_utils as _bu
        _bu.upload_artifacts = lambda tmpdir: tmpdir
    except Exception:
        pass


def _build_program():
    nc = bacc.Bacc(None, target_bir_lowering=False)
    # x pre-tiled on host: [group, k-quad, 128 k, 4 chunks, 512 tokens]; each
    # (group, k-quad) tile is a contiguous 512KB block with 4KB per-partition
    # descriptors, so x DMAs run at HBM line rate
    xT = nc.declare_dram_parameter(
        "xT", [NG, NK // 4, P, 4, GM * P], mybir.dt.bfloat16, isOutput=False
    )
    # W pre-tiled on host: [k-quad, 128 k, 4 chunks * 1024], contiguous 1MB
    # tiles with 8KB per-partition descriptors
    wT = nc.declare_dram_parameter(
        "wT", [NK // 4, P, 4 * D2], mybir.dt.bfloat16, isOutput=False
    )
    ape_rows = nc.declare_dram_parameter("ape_rows", [TC, D], mybir.dt.float32, isOutput=False)
    cache_in = nc.declare_dram_parameter("cache_in", [TC, D2], mybir.dt.float32, isOutput=False)
    out_new = nc.declare_dram_parameter(
        "out_new", [TC, D2], mybir.dt.float32, isOutput=True
    )
    out_pass = nc.declare_dram_parameter(
        "out_pass", [TC, D2], mybir.dt.float32, isOutput=True
    )

    with tile.TileContext(nc) as tc, ExitStack() as ctx:
        wpool = ctx.enter_context(tc.tile_pool(name="w", bufs=NK // 4))
        xpool = ctx.enter_context(tc.tile_pool(name="x", bufs=8))
        opool = ctx.enter_context(tc.tile_pool(name="o", bufs=3))
        apool = ctx.enter_context(tc.tile_pool(name="ape", bufs=2 * GM))
        ppool = ctx.enter_context(tc.tile_pool(name="ps", bufs=8, space="PSUM"))

        # W resident in SBUF: 14 tiles of [128, 4096] bf16 (4 k-chunks each),
        # each a contiguous 1MB DMA. W and x quads are striped across both
        # HWDGE rings in consumption order (each ring carries ~half of W plus
        # ~half of x, ~111GB/s demand each) so early delivery keeps pace with
        # the PE. The first W quad is split per-chunk so matmul 0 starts fast.
        wt = [
            wpool.tile([P, 4 * D2], mybir.dt.bfloat16, tag="w", name=f"w{j}")
            for j in range(NK // 4)
        ]
        for c in range(4):
            nc.sync.dma_start(
                wt[0][:, c * D2:(c + 1) * D2], wT[0, :, c * D2:(c + 1) * D2]
            )

        def w_load(j):
            eng = nc.sync if j % 2 == 0 else nc.scalar
            eng.dma_start(wt[j][:], wT[j])

        # pass-through tile ranges per group (emitted after groups 0..2)
        pass_splits = [(0, 6), (6, 11), (11, 16)]

        # scratch operand for PE warmup matmuls (zeroed: uninitialized SBUF
        # reads fault the exec unit)
        warm_sb = opool.tile([P, 64], mybir.dt.bfloat16, tag="warm", name="warm_sb")
        nc.vector.memset(warm_sb[:], 0.0)

        for g in range(NG):
            psums = [
                ppool.tile([P, D], mybir.dt.float32, tag="acc", name=f"acc{g}_{i}")
                for i in range(GM * 2)
            ]
            if g == 0:
                # Keep the PE busy while the first W/x DMAs are in flight:
                # HAM un-throttles after ~3.4us of sustained activity, so the
                # first real matmuls run at 2.4GHz instead of 1.2GHz. These
                # write psum bank 0, which the first start=True matmul resets.
                for i in range(180):
                    nc.tensor.matmul(
                        psums[0][0:64, 0:64], warm_sb[:, 0:64], warm_sb[:, 0:64],
                        start=True, stop=True,
                    )
            for A in range(NK // 4):
                # x quad-chunk [128 k, 4, 512 tokens]; rings alternate per
                # quad; very first quad split per-chunk for first-MM latency
                xt = xpool.tile([P, 4, GM * P], mybir.dt.bfloat16, tag="x")
                x_eng = nc.scalar if A % 2 == 0 else nc.sync
                if g == 0 and A == 0:
                    for c in range(4):
                        nc.scalar.dma_start(xt[:, c, :], xT[g, A, :, c, :])
                else:
                    x_eng.dma_start(xt[:], xT[g, A])
                if g == 0 and A + 1 < NK // 4:
                    # next W quad, emitted in consumption order on its ring
                    w_load(A + 1)
                for c in range(4):
                    a = 4 * A + c
                    for mi in range(GM):
                        lhsT = xt[:, c, mi * P:(mi + 1) * P]
                        nc.tensor.matmul(
                            psums[2 * mi][:], lhsT, wt[A][:, c * D2:c * D2 + D],
                            start=(a == 0), stop=(a == NK - 1),
                        )
                        nc.tensor.matmul(
                            psums[2 * mi + 1][:], lhsT, wt[A][:, c * D2 + D:(c + 1) * D2],
                            start=(a == 0), stop=(a == NK - 1),
                        )

            # ape rows arrive on the (otherwise idle) SWDGE ring well before
            # eviction; emitted late so Q0 is silent during the startup window
            apes = []
            for mi in range(GM):
                m = g * GM + mi
                at = apool.tile([P, D], mybir.dt.float32, tag="ape", name=f"ape{g}_{mi}")
                nc.gpsimd.dma_start(at[:], ape_rows[m * P:(m + 1) * P, :])
                apes.append(at)

            for mi in range(GM):
                m = g * GM + mi
                ot = opool.tile([P, D2], mybir.dt.float32, tag="o", name=f"ot{g}_{mi}")
                nc.vector.tensor_copy(ot[:, 0:D], psums[2 * mi][:])
                nc.vector.tensor_add(ot[:, D:D2], psums[2 * mi + 1][:], apes[mi][:])
                # sync ring is idle once W is resident; stores complete fast
                nc.sync.dma_start(out_new[m * P:(m + 1) * P, :], ot[:])

            # pass-through of untouched cache blocks, bounced through the
            # eviction tile pool: the slot dependency chains each piece
            # behind this group's stores, keeping it out of the startup
            # window without fake timing hints
            if g < 3:
                for i in range(*pass_splits[g]):
                    bt = opool.tile([P, D2], mybir.dt.float32, tag="o",
                                    name=f"pt{g}_{i}")
                    nc.gpsimd.dma_start(bt[:], cache_in[i * P:(i + 1) * P, :])
                    nc.gpsimd.dma_start(out_pass[i * P:(i + 1) * P, :], bt[:])

    nc.compile()
    return nc


def _get_program():
    global _PROGRAM
    if _PROGRAM is None:
        _install_ntff_hook()
        _PROGRAM = _build_program()
    return _PROGRAM


def kernel(x, W, ape, state_cache, positions, slot_mapping, block_size=8):
    global LAST_RESULTS
    x = np.asarray(x)
    W = np.asarray(W)
    ape = np.asarray(ape)
    state_cache = np.asarray(state_cache)
    positions = np.asarray(positions)
    slot_mapping = np.asarray(slot_mapping)

    assert x.shape == (T, H) and W.shape == (D2, H) and ape.shape == (CR, D)
    assert state_cache.shape == (4096, 8, D2)

    # host-side input prep (layout/sharding glue)
    # W^T repacked to [14, 128, 4096]: tile j partition p holds rows
    # (4j+c)*128+p of W^T for c=0..3
    wTb = np.ascontiguousarray(
        W.astype(BF16).T.reshape(NK // 4, 4, P, D2).transpose(0, 2, 1, 3)
        .reshape(NK // 4, P, 4 * D2)
    )
    xb = x.astype(BF16)                                     # [T, H] bf16
    pos_mod = (positions.astype(np.int64) % CR).astype(np.int64)
    ape_rows_full = np.ascontiguousarray(ape[pos_mod])      # [T, D] f32
    cache_flat = state_cache.reshape(NB, D2)

    fast = (
        slot_mapping.shape == (T,)
        and np.array_equal(slot_mapping, np.arange(T, dtype=slot_mapping.dtype))
    )

    zeros_cache = None if fast else np.zeros((TC, D2), np.float32)
    in_maps = []
    for c in range(NCORES):
        t0, t1 = c * TC, (c + 1) * TC
        in_maps.append({
            # [NG, 14, 128, 4, 512]: per-(group, k-quad) contiguous tiles
            "xT": np.ascontiguousarray(
                xb[t0:t1].reshape(NG, GM * P, NK // 4, 4, P)
                .transpose(0, 2, 4, 3, 1)
            ),
            "wT": wTb,
            "ape_rows": ape_rows_full[t0:t1],
            "cache_in": (
                np.ascontiguousarray(cache_flat[T + t0:T + t1]).astype(
                    np.float32, copy=False
                )
                if fast else zeros_cache
            ),
        })

    nc = _get_program()
    trace = os.environ.get("KERNEL_TRACE", "0") == "1"
    res = run_bass_kernel_spmd(nc, in_maps, list(range(NCORES)), trace=trace)
    LAST_RESULTS = res

    out_flat = np.empty((NB, D2), np.float32)
    if fast:
        for c in range(NCORES):
            t0, t1 = c * TC, (c + 1) * TC
            out_flat[t0:t1] = np.asarray(res.results[c]["out_new"])
            out_flat[T + t0:T + t1] = np.asarray(res.results[c]["out_pass"])
    else:
        # general slot_mapping: device computes new_vals; host scatters
        out_flat[:] = cache_flat
        new_vals = np.concatenate(
            [np.asarray(res.results[c]["out_new"]) for c in range(NCORES)], axis=0
        )
        ok = (slot_mapping >= 0) & (slot_mapping < NB)
        out_flat[slot_mapping[ok]] = new_vals[ok]
    return out_flat.reshape(4096, 8, D2)


# revision 30
# speedup vs baseline: 1.0078x; 1.0075x over previous
"""Trainium2 Bass kernel for nn_DeepseekCompressor (scatter_memory).

Computation: kv_score = x @ W.T; score half += ape[positions % 128];
rows scattered into a paged state cache at slot_mapping.

Sharding (8 NeuronCores, data-parallel over tokens):
  - x, positions, slot_mapping sharded by token (2048 tokens/core).
  - W, ape replicated (host pre-transposes W and casts to bf16; ape rows
    pre-gathered per token in f32 on host).
  - state_cache sharded by block: with the contiguous slot_mapping each
    core's tokens land in its own contiguous range of cache rows, and the
    untouched half of the cache is pass-through-copied, one slice per core.

Device kernel per core: bf16 GEMM [2048,7168]@[7168,1024] -> f32 PSUM,
ape added at PSUM eviction (free: replaces the copy), contiguous DMA
scatter into the cache rows, DRAM->DRAM pass-through of untouched blocks.
"""

import os
import sys
import types
from contextlib import ExitStack

if "/opt/trn_rl_repo" not in sys.path:
    sys.path.insert(0, "/opt/trn_rl_repo")

import numpy as np
import ml_dtypes

import concourse.bass as bass
import concourse.tile as tile
from concourse import bacc, mybir
from concourse.bass_utils import run_bass_kernel_spmd

NCORES = 8
T = 16384          # tokens
H = 7168           # hidden
D2 = 1024          # 2 * state_width
D = 512            # state_width
CR = 128           # compress ratio (ape rows)
TC = T // NCORES   # tokens per core
P = 128
NK = H // P        # k-chunks of 128
MT = TC // P       # m-tiles per core (16)
GM = 4             # m-tiles per PSUM group
NG = MT // GM      # groups (4)
NB = 4096 * 8      # flat cache rows

BF16 = ml_dtypes.bfloat16

LAST_RESULTS = None
_PROGRAM = None


def _install_ntff_hook():
    """Make trace=True work under axon: register the NTFF profile hook that
    the image's antenv is missing, and stub the (egress-only) artifact
    upload. No-ops if anything is unavailable."""
    try:
        import antenv
        if "antenv.axon_hooks" not in sys.modules:
            mod = types.ModuleType("antenv.axon_hooks")
            _state = {"hook": None}
            mod.set_axon_ntff_profile_hook = lambda h: _state.__setitem__("hook", h)
            mod.get_axon_ntff_profile_hook = lambda: _state["hook"]
            sys.modules["antenv.axon_hooks"] = mod
            antenv.axon_hooks = mod
            from trn_agent_boot.trn_boot import _ntff_profile_via_ctypes
            mod.set_axon_ntff_profile_hook(
                _ntff_profile_via_ctypes("/opt/axon/libaxon_pjrt.so")
            )
        import concourse.bass_utils as _bu
        _bu.upload_artifacts = lambda tmpdir: tmpdir
    except Exception:
        pass


def _build_program():
    nc = bacc.Bacc(None, target_bir_lowering=False)
    # x pre-tiled on host: [group, k-quad, 128 k, 4 chunks, 512 tokens]; each
    # (group, k-quad) tile is a contiguous 512KB block with 4KB per-partition
    # descriptors, so x DMAs run at HBM line rate
    xT = nc.declare_dram_parameter(
        "xT", [NG, NK // 4, P, 4, GM * P], mybir.dt.bfloat16, isOutput=False
    )
    # W pre-tiled on host: [k-quad, 128 k, 4 chunks * 1024], contiguous 1MB
    # tiles with 8KB per-partition descriptors
    wT = nc.declare_dram_parameter(
        "wT", [NK // 4, P, 4 * D2], mybir.dt.bfloat16, isOutput=False
    )
    ape_rows = nc.declare_dram_parameter("ape_rows", [TC, D], mybir.dt.float32, isOutput=False)
    cache_in = nc.declare_dram_parameter("cache_in", [TC, D2], mybir.dt.float32, isOutput=False)
    out_new = nc.declare_dram_parameter(
        "out_new", [TC, D2], mybir.dt.float32, isOutput=True
    )
    out_pass = nc.declare_dram_parameter(
        "out_pass", [TC, D2], mybir.dt.float32, isOutput=True
    )

    with tile.TileContext(nc) as tc, ExitStack() as ctx:
        wpool = ctx.enter_context(tc.tile_pool(name="w", bufs=NK // 4))
        xpool = ctx.enter_context(tc.tile_pool(name="x", bufs=8))
        opool = ctx.enter_context(tc.tile_pool(name="o", bufs=3))
        apool = ctx.enter_context(tc.tile_pool(name="ape", bufs=2 * GM))
        ppool = ctx.enter_context(tc.tile_pool(name="ps", bufs=8, space="PSUM"))

        # W resident in SBUF: 14 tiles of [128, 4096] bf16 (4 k-chunks each),
        # each a contiguous 1MB DMA. W and x quads are striped across both
        # HWDGE rings in consumption order (each ring carries ~half of W plus
        # ~half of x, ~111GB/s demand each) so early delivery keeps pace with
        # the PE. The first W quad is split per-chunk so matmul 0 starts fast.
        wt = [
            wpool.tile([P, 4 * D2], mybir.dt.bfloat16, tag="w", name=f"w{j}")
            for j in range(NK // 4)
        ]
        for c in range(4):
            nc.sync.dma_start(
                wt[0][:, c * D2:(c + 1) * D2], wT[0, :, c * D2:(c + 1) * D2]
            )

        def w_load(j):
            eng = nc.sync if j % 2 == 0 else nc.scalar
            eng.dma_start(wt[j][:], wT[j])

        # pass-through tile ranges per group (emitted after groups 0..2)
        pass_splits = [(0, 6), (6, 11), (11, 16)]

        # scratch operand for PE warmup matmuls (zeroed: uninitialized SBUF
        # reads fault the exec unit)
        warm_sb = opool.tile([P, 64], mybir.dt.bfloat16, tag="warm", name="warm_sb")
        nc.vector.memset(warm_sb[:], 0.0)

        for g in range(NG):
            psums = [
                ppool.tile([P, D], mybir.dt.float32, tag="acc", name=f"acc{g}_{i}")
                for i in range(GM * 2)
            ]
            if g == 0:
                # Keep the PE busy while the first W/x DMAs are in flight:
                # HAM un-throttles after ~3.4us of sustained activity, so the
                # first real matmuls run at 2.4GHz instead of 1.2GHz. These
                # write psum bank 0, which the first start=True matmul resets.
                for i in range(140):
                    nc.tensor.matmul(
                        psums[0][0:64, 0:64], warm_sb[:, 0:64], warm_sb[:, 0:64],
                        start=True, stop=True,
                    )
            for A in range(NK // 4):
                # x quad-chunk [128 k, 4, 512 tokens]; rings alternate per
                # quad; very first quad split per-chunk for first-MM latency
                xt = xpool.tile([P, 4, GM * P], mybir.dt.bfloat16, tag="x")
                x_eng = nc.scalar if A % 2 == 0 else nc.sync
                if g == 0 and A == 0:
                    for c in range(4):
                        nc.scalar.dma_start(xt[:, c, :], xT[g, A, :, c, :])
                else:
                    x_eng.dma_start(xt[:], xT[g, A])
                if g == 0 and A + 1 < NK // 4:
                    # next W quad, emitted in consumption order on its ring
                    w_load(A + 1)
                for c in range(4):
                    a = 4 * A + c
                    for mi in range(GM):
                        lhsT = xt[:, c, mi * P:(mi + 1) * P]
                        nc.tensor.matmul(
                            psums[2 * mi][:], lhsT, wt[A][:, c * D2:c * D2 + D],
                            start=(a == 0), stop=(a == NK - 1),
                        )
                        nc.tensor.matmul(
                            psums[2 * mi + 1][:], lhsT, wt[A][:, c * D2 + D:(c + 1) * D2],
                            start=(a == 0), stop=(a == NK - 1),
                        )

            # ape rows arrive on the (otherwise idle) SWDGE ring well before
            # eviction; emitted late so Q0 is silent during the startup window
            apes = []
            for mi in range(GM):
                m = g * GM + mi
                at = apool.tile([P, D], mybir.dt.float32, tag="ape", name=f"ape{g}_{mi}")
                nc.gpsimd.dma_start(at[:], ape_rows[m * P:(m + 1) * P, :])
                apes.append(at)

            for mi in range(GM):
                m = g * GM + mi
                ot = opool.tile([P, D2], mybir.dt.float32, tag="o", name=f"ot{g}_{mi}")
                nc.vector.tensor_copy(ot[:, 0:D], psums[2 * mi][:])
                nc.vector.tensor_add(ot[:, D:D2], psums[2 * mi + 1][:], apes[mi][:])
                # sync ring is idle once W is resident; stores complete fast
                nc.sync.dma_start(out_new[m * P:(m + 1) * P, :], ot[:])

            # pass-through of untouched cache blocks, bounced through the
            # eviction tile pool: the slot dependency chains each piece
            # behind this group's stores, keeping it out of the startup
            # window without fake timing hints
            if g < 3:
                for i in range(*pass_splits[g]):
                    bt = opool.tile([P, D2], mybir.dt.float32, tag="o",
                                    name=f"pt{g}_{i}")
                    nc.gpsimd.dma_start(bt[:], cache_in[i * P:(i + 1) * P, :])
                    nc.gpsimd.dma_start(out_pass[i * P:(i + 1) * P, :], bt[:])

    nc.compile()
    return nc


def _get_program():
    global _PROGRAM
    if _PROGRAM is None:
        _install_ntff_hook()
        _PROGRAM = _build_program()
    return _PROGRAM


def kernel(x, W, ape, state_cache, positions, slot_mapping, block_size=8):
    global LAST_RESULTS
    x = np.asarray(x)
    W = np.asarray(W)
    ape = np.asarray(ape)
    state_cache = np.asarray(state_cache)
    positions = np.asarray(positions)
    slot_mapping = np.asarray(slot_mapping)

    assert x.shape == (T, H) and W.shape == (D2, H) and ape.shape == (CR, D)
    assert state_cache.shape == (4096, 8, D2)

    # host-side input prep (layout/sharding glue)
    # W^T repacked to [14, 128, 4096]: tile j partition p holds rows
    # (4j+c)*128+p of W^T for c=0..3
    wTb = np.ascontiguousarray(
        W.astype(BF16).T.reshape(NK // 4, 4, P, D2).transpose(0, 2, 1, 3)
        .reshape(NK // 4, P, 4 * D2)
    )
    xb = x.astype(BF16)                                     # [T, H] bf16
    pos_mod = (positions.astype(np.int64) % CR).astype(np.int64)
    ape_rows_full = np.ascontiguousarray(ape[pos_mod])      # [T, D] f32
    cache_flat = state_cache.reshape(NB, D2)

    fast = (
        slot_mapping.shape == (T,)
        and np.array_equal(slot_mapping, np.arange(T, dtype=slot_mapping.dtype))
    )

    zeros_cache = None if fast else np.zeros((TC, D2), np.float32)
    in_maps = []
    for c in range(NCORES):
        t0, t1 = c * TC, (c + 1) * TC
        in_maps.append({
            # [NG, 14, 128, 4, 512]: per-(group, k-quad) contiguous tiles
            "xT": np.ascontiguousarray(
                xb[t0:t1].reshape(NG, GM * P, NK // 4, 4, P)
                .transpose(0, 2, 4, 3, 1)
            ),
            "wT": wTb,
            "ape_rows": ape_rows_full[t0:t1],
            "cache_in": (
                np.ascontiguousarray(cache_flat[T + t0:T + t1]).astype(
                    np.float32, copy=False
                )
                if fast else zeros_cache
            ),
        })

    nc = _get_program()
    trace = os.environ.get("KERNEL_TRACE", "0") == "1"
    res = run_bass_kernel_spmd(nc, in_maps, list(range(NCORES)), trace=trace)
    LAST_RESULTS = res

    out_flat = np.empty((NB, D2), np.float32)
    if fast:
        for c in range(NCORES):
            t0, t1 = c * TC, (c + 1) * TC
            out_flat[t0:t1] = np.asarray(res.results[c]["out_new"])
            out_flat[T + t0:T + t1] = np.asarray(res.results[c]["out_pass"])
    else:
        # general slot_mapping: device computes new_vals; host scatters
        out_flat[:] = cache_flat
        new_vals = np.concatenate(
            [np.asarray(res.results[c]["out_new"]) for c in range(NCORES)], axis=0
        )
        ok = (slot_mapping >= 0) & (slot_mapping < NB)
        out_flat[slot_mapping[ok]] = new_vals[ok]
    return out_flat.reshape(4096, 8, D2)


# revision 31
# speedup vs baseline: 1.0101x; 1.0022x over previous
"""Trainium2 Bass kernel for nn_DeepseekCompressor (scatter_memory).

Computation: kv_score = x @ W.T; score half += ape[positions % 128];
rows scattered into a paged state cache at slot_mapping.

Sharding (8 NeuronCores, data-parallel over tokens):
  - x, positions, slot_mapping sharded by token (2048 tokens/core).
  - W, ape replicated (host pre-transposes W and casts to bf16; ape rows
    pre-gathered per token in f32 on host).
  - state_cache sharded by block: with the contiguous slot_mapping each
    core's tokens land in its own contiguous range of cache rows, and the
    untouched half of the cache is pass-through-copied, one slice per core.

Device kernel per core: bf16 GEMM [2048,7168]@[7168,1024] -> f32 PSUM,
ape added at PSUM eviction (free: replaces the copy-back), contiguous DMA
scatter into the cache rows, and an SBUF-bounced pass-through of the
untouched blocks scheduled behind each group's stores so it stays out of
the startup bandwidth window. W is SBUF-resident; W/x loads are striped
across both HWDGE rings in consumption order; dummy matmuls warm the PE
clock (HAM) while the first DMAs are in flight. Measured ~420us on HW
(PE busy ~391us = bf16 matmul roofline for this shape).
"""

import os
import sys
import types
from contextlib import ExitStack

if "/opt/trn_rl_repo" not in sys.path:
    sys.path.insert(0, "/opt/trn_rl_repo")

import numpy as np
import ml_dtypes

import concourse.bass as bass
import concourse.tile as tile
from concourse import bacc, mybir
from concourse.bass_utils import run_bass_kernel_spmd

NCORES = 8
T = 16384          # tokens
H = 7168           # hidden
D2 = 1024          # 2 * state_width
D = 512            # state_width
CR = 128           # compress ratio (ape rows)
TC = T // NCORES   # tokens per core
P = 128
NK = H // P        # k-chunks of 128
MT = TC // P       # m-tiles per core (16)
GM = 4             # m-tiles per PSUM group
NG = MT // GM      # groups (4)
NB = 4096 * 8      # flat cache rows

BF16 = ml_dtypes.bfloat16

LAST_RESULTS = None
_PROGRAM = None


def _install_ntff_hook():
    """Make trace=True work under axon: register the NTFF profile hook that
    the image's antenv is missing, and stub the (egress-only) artifact
    upload. No-ops if anything is unavailable."""
    try:
        import antenv
        if "antenv.axon_hooks" not in sys.modules:
            mod = types.ModuleType("antenv.axon_hooks")
            _state = {"hook": None}
            mod.set_axon_ntff_profile_hook = lambda h: _state.__setitem__("hook", h)
            mod.get_axon_ntff_profile_hook = lambda: _state["hook"]
            sys.modules["antenv.axon_hooks"] = mod
            antenv.axon_hooks = mod
            from trn_agent_boot.trn_boot import _ntff_profile_via_ctypes
            mod.set_axon_ntff_profile_hook(
                _ntff_profile_via_ctypes("/opt/axon/libaxon_pjrt.so")
            )
        import concourse.bass_utils as _bu
        _bu.upload_artifacts = lambda tmpdir: tmpdir
    except Exception:
        pass


def _build_program():
    nc = bacc.Bacc(None, target_bir_lowering=False)
    # x pre-tiled on host: [group, k-quad, 128 k, 4 chunks, 512 tokens]; each
    # (group, k-quad) tile is a contiguous 512KB block with 4KB per-partition
    # descriptors, so x DMAs run at HBM line rate
    xT = nc.declare_dram_parameter(
        "xT", [NG, NK // 4, P, 4, GM * P], mybir.dt.bfloat16, isOutput=False
    )
    # W pre-tiled on host: [k-quad, 128 k, 4 chunks * 1024], contiguous 1MB
    # tiles with 8KB per-partition descriptors
    wT = nc.declare_dram_parameter(
        "wT", [NK // 4, P, 4 * D2], mybir.dt.bfloat16, isOutput=False
    )
    ape_rows = nc.declare_dram_parameter("ape_rows", [TC, D], mybir.dt.float32, isOutput=False)
    cache_in = nc.declare_dram_parameter("cache_in", [TC, D2], mybir.dt.float32, isOutput=False)
    out_new = nc.declare_dram_parameter(
        "out_new", [TC, D2], mybir.dt.float32, isOutput=True
    )
    out_pass = nc.declare_dram_parameter(
        "out_pass", [TC, D2], mybir.dt.float32, isOutput=True
    )

    with tile.TileContext(nc) as tc, ExitStack() as ctx:
        wpool = ctx.enter_context(tc.tile_pool(name="w", bufs=NK // 4))
        xpool = ctx.enter_context(tc.tile_pool(name="x", bufs=8))
        opool = ctx.enter_context(tc.tile_pool(name="o", bufs=3))
        apool = ctx.enter_context(tc.tile_pool(name="ape", bufs=2 * GM))
        ppool = ctx.enter_context(tc.tile_pool(name="ps", bufs=8, space="PSUM"))

        # W resident in SBUF: 14 tiles of [128, 4096] bf16 (4 k-chunks each),
        # each a contiguous 1MB DMA. W and x quads are striped across both
        # HWDGE rings in consumption order (each ring carries ~half of W plus
        # ~half of x, ~111GB/s demand each) so early delivery keeps pace with
        # the PE. The first W quad is split per-chunk so matmul 0 starts fast.
        wt = [
            wpool.tile([P, 4 * D2], mybir.dt.bfloat16, tag="w", name=f"w{j}")
            for j in range(NK // 4)
        ]
        for c in range(4):
            nc.sync.dma_start(
                wt[0][:, c * D2:(c + 1) * D2], wT[0, :, c * D2:(c + 1) * D2]
            )

        def w_load(j):
            eng = nc.sync if j % 2 == 0 else nc.scalar
            eng.dma_start(wt[j][:], wT[j])

        # pass-through tile ranges per group (emitted after groups 0..2)
        pass_splits = [(0, 6), (6, 11), (11, 16)]

        # scratch operand for PE warmup matmuls (zeroed: uninitialized SBUF
        # reads fault the exec unit)
        warm_sb = opool.tile([P, 64], mybir.dt.bfloat16, tag="warm", name="warm_sb")
        nc.vector.memset(warm_sb[:], 0.0)

        for g in range(NG):
            psums = [
                ppool.tile([P, D], mybir.dt.float32, tag="acc", name=f"acc{g}_{i}")
                for i in range(GM * 2)
            ]
            if g == 0:
                # Keep the PE busy while the first W/x DMAs are in flight:
                # HAM un-throttles after ~3.4us of sustained activity, so the
                # first real matmuls run at 2.4GHz instead of 1.2GHz. These
                # write psum bank 0, which the first start=True matmul resets.
                for i in range(140):
                    nc.tensor.matmul(
                        psums[0][0:64, 0:64], warm_sb[:, 0:64], warm_sb[:, 0:64],
                        start=True, stop=True,
                    )
            for A in range(NK // 4):
                # x quad-chunk [128 k, 4, 512 tokens]; rings alternate per
                # quad; very first quad split per-chunk for first-MM latency
                xt = xpool.tile([P, 4, GM * P], mybir.dt.bfloat16, tag="x")
                x_eng = nc.scalar if A % 2 == 0 else nc.sync
                if g == 0 and A == 0:
                    for c in range(4):
                        nc.scalar.dma_start(xt[:, c, :], xT[g, A, :, c, :])
                else:
                    x_eng.dma_start(xt[:], xT[g, A])
                if g == 0 and A + 1 < NK // 4:
                    # next W quad, emitted in consumption order on its ring
                    w_load(A + 1)
                for c in range(4):
                    a = 4 * A + c
                    for mi in range(GM):
                        lhsT = xt[:, c, mi * P:(mi + 1) * P]
                        nc.tensor.matmul(
                            psums[2 * mi][:], lhsT, wt[A][:, c * D2:c * D2 + D],
                            start=(a == 0), stop=(a == NK - 1),
                        )
                        nc.tensor.matmul(
                            psums[2 * mi + 1][:], lhsT, wt[A][:, c * D2 + D:(c + 1) * D2],
                            start=(a == 0), stop=(a == NK - 1),
                        )

            # ape rows arrive on the (otherwise idle) SWDGE ring well before
            # eviction; emitted late so Q0 is silent during the startup window
            apes = []
            for mi in range(GM):
                m = g * GM + mi
                at = apool.tile([P, D], mybir.dt.float32, tag="ape", name=f"ape{g}_{mi}")
                nc.gpsimd.dma_start(at[:], ape_rows[m * P:(m + 1) * P, :])
                apes.append(at)

            for mi in range(GM):
                m = g * GM + mi
                ot = opool.tile([P, D2], mybir.dt.float32, tag="o", name=f"ot{g}_{mi}")
                nc.vector.tensor_copy(ot[:, 0:D], psums[2 * mi][:])
                nc.vector.tensor_add(ot[:, D:D2], psums[2 * mi + 1][:], apes[mi][:])
                # sync ring is idle once W is resident; stores complete fast
                nc.sync.dma_start(out_new[m * P:(m + 1) * P, :], ot[:])

            # pass-through of untouched cache blocks, bounced through the
            # eviction tile pool: the slot dependency chains each piece
            # behind this group's stores, keeping it out of the startup
            # window without fake timing hints
            if g < 3:
                for i in range(*pass_splits[g]):
                    bt = opool.tile([P, D2], mybir.dt.float32, tag="o",
                                    name=f"pt{g}_{i}")
                    nc.gpsimd.dma_start(bt[:], cache_in[i * P:(i + 1) * P, :])
                    nc.gpsimd.dma_start(out_pass[i * P:(i + 1) * P, :], bt[:])

    nc.compile()
    return nc


def _get_program():
    global _PROGRAM
    if _PROGRAM is None:
        _install_ntff_hook()
        _PROGRAM = _build_program()
    return _PROGRAM


def kernel(x, W, ape, state_cache, positions, slot_mapping, block_size=8):
    global LAST_RESULTS
    x = np.asarray(x)
    W = np.asarray(W)
    ape = np.asarray(ape)
    state_cache = np.asarray(state_cache)
    positions = np.asarray(positions)
    slot_mapping = np.asarray(slot_mapping)

    assert x.shape == (T, H) and W.shape == (D2, H) and ape.shape == (CR, D)
    assert state_cache.shape == (4096, 8, D2)

    # host-side input prep (layout/sharding glue)
    # W^T repacked to [14, 128, 4096]: tile j partition p holds rows
    # (4j+c)*128+p of W^T for c=0..3
    wTb = np.ascontiguousarray(
        W.astype(BF16).T.reshape(NK // 4, 4, P, D2).transpose(0, 2, 1, 3)
        .reshape(NK // 4, P, 4 * D2)
    )
    xb = x.astype(BF16)                                     # [T, H] bf16
    pos_mod = (positions.astype(np.int64) % CR).astype(np.int64)
    ape_rows_full = np.ascontiguousarray(ape[pos_mod])      # [T, D] f32
    cache_flat = state_cache.reshape(NB, D2)

    fast = (
        slot_mapping.shape == (T,)
        and np.array_equal(slot_mapping, np.arange(T, dtype=slot_mapping.dtype))
    )

    zeros_cache = None if fast else np.zeros((TC, D2), np.float32)
    in_maps = []
    for c in range(NCORES):
        t0, t1 = c * TC, (c + 1) * TC
        in_maps.append({
            # [NG, 14, 128, 4, 512]: per-(group, k-quad) contiguous tiles
            "xT": np.ascontiguousarray(
                xb[t0:t1].reshape(NG, GM * P, NK // 4, 4, P)
                .transpose(0, 2, 4, 3, 1)
            ),
            "wT": wTb,
            "ape_rows": ape_rows_full[t0:t1],
            "cache_in": (
                np.ascontiguousarray(cache_flat[T + t0:T + t1]).astype(
                    np.float32, copy=False
                )
                if fast else zeros_cache
            ),
        })

    nc = _get_program()
    trace = os.environ.get("KERNEL_TRACE", "0") == "1"
    res = run_bass_kernel_spmd(nc, in_maps, list(range(NCORES)), trace=trace)
    LAST_RESULTS = res

    out_flat = np.empty((NB, D2), np.float32)
    if fast:
        for c in range(NCORES):
            t0, t1 = c * TC, (c + 1) * TC
            out_flat[t0:t1] = np.asarray(res.results[c]["out_new"])
            out_flat[T + t0:T + t1] = np.asarray(res.results[c]["out_pass"])
    else:
        # general slot_mapping: device computes new_vals; host scatters
        out_flat[:] = cache_flat
        new_vals = np.concatenate(
            [np.asarray(res.results[c]["out_new"]) for c in range(NCORES)], axis=0
        )
        ok = (slot_mapping >= 0) & (slot_mapping < NB)
        out_flat[slot_mapping[ok]] = new_vals[ok]
    return out_flat.reshape(4096, 8, D2)
